# revision 12
# baseline (speedup 1.0000x reference)
"""Trainium2 Bass kernel for MultiHeadAttention with relative positional embeddings.

Sharding: 8 cores = 4 batches x 2 head-groups (8 heads each).
Per core: column-sharded QKV projections, per-head causal attention with
Transformer-XL style relative position terms, row-sharded output projection.
Host sums the two partial outputs per batch.

Relative-shift terms are computed with plain matmuls plus diagonally
re-strided DMA views of DRAM bounce buffers (row stride R-1 / R+1 instead of
R), with the causal mask folded into a -1e9 pad region of the bias buffer.
"""

import os
import sys

sys.path.insert(0, "/opt/trn_rl_repo")

import ml_dtypes
import numpy as np

import concourse.bass as bass
import concourse.tile as tile
from concourse import bacc, mybir
from concourse.bass_utils import run_bass_kernel_spmd

BF16 = mybir.dt.bfloat16
F32 = mybir.dt.float32

B, L, D, H, HD = 4, 1024, 1024, 16, 64
HLOC = 8          # heads per core
OD = HLOC * HD    # 512, local projection width
NT = L // 128     # 8 q-tiles
RS = 2048         # bounce-buffer row stride, elements
NEG = -1.0e9
SCALE = 1.0 / (HD ** 0.5)  # 1/8

_CACHE = {}
run_info = {}


def _chunks(w, step=512):
    out, c = [], 0
    while c < w:
        out.append((c, min(step, w - c)))
        c += step
    return out


def _build():
    nc = bacc.Bacc("TRN2", target_bir_lowering=False, debug=False)

    xt = nc.dram_tensor("xt", [D, L], BF16, kind="ExternalInput")
    wqt = nc.dram_tensor("wqt", [D, OD], BF16, kind="ExternalInput")
    wkt = nc.dram_tensor("wkt", [D, OD], BF16, kind="ExternalInput")
    wvt = nc.dram_tensor("wvt", [D, OD], BF16, kind="ExternalInput")
    wot = nc.dram_tensor("wot", [OD, D], BF16, kind="ExternalInput")
    ekt = nc.dram_tensor("ekt", [128, L], BF16, kind="ExternalInput")
    e2 = nc.dram_tensor("e2", [L, HD], BF16, kind="ExternalInput")
    iden = nc.dram_tensor("iden", [128, 128], BF16, kind="ExternalInput")
    outp = nc.dram_tensor("outp", [L, D], F32, kind="ExternalOutput")

    # ping-pong bounce buffers: [pair parity][head]
    apads = [nc.dram_tensor(f"apad{i}", [L * RS], BF16) for i in range(4)]
    ppads = [nc.dram_tensor(f"ppad{i}", [L * RS], BF16) for i in range(4)]

    from contextlib import ExitStack

    with tile.TileContext(nc) as tc, ExitStack() as ctx:
        const = ctx.enter_context(tc.tile_pool(name="const", bufs=1))

        # ---- persistent SBUF state ----
        wo_sb = const.tile([128, OD // 128, D], BF16)
        nc.sync.dma_start(wo_sb[:], wot.ap().rearrange("(a p) m -> p a m", p=128))
        qt_sb = const.tile([128, 4, L], BF16)
        kt_sb = const.tile([128, 4, L], BF16)
        v_sb = const.tile([128, 8, OD], BF16)
        ek_sb = const.tile([128, L], BF16)
        nc.sync.dma_start(ek_sb[:], ekt.ap())
        e2_sb = const.tile([128, 8, HD], BF16)
        nc.sync.dma_start(e2_sb[:], e2.ap().rearrange("(a p) d -> p a d", p=128))
        id_sb = const.tile([128, 128], BF16)
        nc.sync.dma_start(id_sb[:], iden.ap())
        ota_sb = const.tile([128, 4, L], BF16)  # attention out.T, [od, tok]

        # ---- one-time pad init of bounce buffers ----
        with tc.tile_pool(name="padinit", bufs=1) as padp:
            msk = padp.tile([128, 1024], BF16)
            nc.vector.memset(msk[:], NEG)
            zer = padp.tile([128, 1024], BF16)
            nc.vector.memset(zer[:], 0.0)
            for i in range(4):
                # mask cols [1024, 1152) of every row (causal mask via bias read)
                nc.sync.dma_start(
                    bass.AP(apads[i], 1024, [[8 * RS, 128], [RS, 8], [1, 128]]),
                    msk[:],
                )
                # zero cols [896, 1024) of every row (left-pad for diagonal P read)
                nc.sync.dma_start(
                    bass.AP(ppads[i], 896, [[8 * RS, 128], [RS, 8], [1, 128]]),
                    zer[:],
                )

        # ---- projections ----
        with (
            tc.tile_pool(name="xw", bufs=1) as xw,
            tc.tile_pool(name="projp", bufs=4, space="PSUM") as projp,
        ):
            xt_sb = xw.tile([128, 8, L], BF16)
            wq_sb = xw.tile([128, 8, OD], BF16)
            wk_sb = xw.tile([128, 8, OD], BF16)
            wv_sb = xw.tile([128, 8, OD], BF16)
            # per-dm-tile loads so the first matmuls can start early
            for i in range(8):
                nc.sync.dma_start(
                    xt_sb[:, i, :],
                    xt.ap().rearrange("(a p) t -> p a t", p=128)[:, i, :],
                )
                nc.sync.dma_start(
                    wq_sb[:, i, :],
                    wqt.ap().rearrange("(a p) o -> p a o", p=128)[:, i, :],
                )
                nc.sync.dma_start(
                    wk_sb[:, i, :],
                    wkt.ap().rearrange("(a p) o -> p a o", p=128)[:, i, :],
                )
                nc.sync.dma_start(
                    wv_sb[:, i, :],
                    wvt.ap().rearrange("(a p) o -> p a o", p=128)[:, i, :],
                )

            ncopy = 0
            for dst, w_sb in ((qt_sb, wq_sb), (kt_sb, wk_sb)):
                for a in range(4):
                    for c in range(2):
                        ps = projp.tile([128, 512], F32)
                        for i in range(8):
                            nc.tensor.matmul(
                                ps[:],
                                w_sb[:, i, a * 128:(a + 1) * 128],
                                xt_sb[:, i, c * 512:(c + 1) * 512],
                                start=(i == 0),
                                stop=(i == 7),
                            )
                        eng = nc.vector if ncopy % 2 == 0 else nc.scalar
                        if eng is nc.vector:
                            eng.tensor_copy(dst[:, a, c * 512:(c + 1) * 512], ps[:])
                        else:
                            eng.copy(dst[:, a, c * 512:(c + 1) * 512], ps[:])
                        ncopy += 1
            for tt in range(8):
                ps = projp.tile([128, 512], F32)
                for i in range(8):
                    nc.tensor.matmul(
                        ps[:],
                        xt_sb[:, i, tt * 128:(tt + 1) * 128],
                        wv_sb[:, i, :],
                        start=(i == 0),
                        stop=(i == 7),
                    )
                eng = nc.vector if ncopy % 2 == 0 else nc.scalar
                if eng is nc.vector:
                    eng.tensor_copy(v_sb[:, tt, :], ps[:])
                else:
                    eng.copy(v_sb[:, tt, :], ps[:])
                ncopy += 1

        # ---- attention, one head-pair per iteration ----
        attn_ctx = ctx.enter_context(ExitStack())
        pep = attn_ctx.enter_context(tc.tile_pool(name="pep", bufs=2, space="PSUM"))
        trp = attn_ctx.enter_context(tc.tile_pool(name="trp", bufs=2, space="PSUM"))
        otp = attn_ctx.enter_context(tc.tile_pool(name="otp", bufs=1, space="PSUM"))
        sbw = attn_ctx.enter_context(tc.tile_pool(name="sbw", bufs=2))
        strips = attn_ctx.enter_context(tc.tile_pool(name="strips", bufs=1))

        for hp in range(4):
            par = hp % 2
            # strips[x][:, t, u, :] holds tile (t-qtile, u-ktile/jtile) transposed
            pt_s = [strips.tile([128, 8, 8, 128], BF16, tag=f"pt{h}", name=f"pt{h}") for h in range(2)]
            pdt_s = [strips.tile([128, 8, 8, 128], BF16, tag=f"pdt{h}", name=f"pdt{h}") for h in range(2)]

            # --- stage A: QE = Q @ E^T, written to apad rows ---
            for t in range(NT):
                wj = 128 * (t + 1)
                j0 = L - wj
                for hi, bp in enumerate((0, 64)):
                    apad = apads[par * 2 + hi]
                    ps = pep.tile([128, 1024], F32, tag="mm", name="psqe")
                    for (c0, w) in _chunks(wj):
                        nc.tensor.matmul(
                            ps[:, c0:c0 + w],
                            qt_sb[bp:bp + 64, hp, t * 128:(t + 1) * 128],
                            ek_sb[bp:bp + 64, j0 + c0:j0 + c0 + w],
                            start=True,
                            stop=True,
                        )
                    qe = sbw.tile([128, 1024], BF16, tag="qe")
                    if (t + hi) % 2 == 0:
                        nc.vector.tensor_copy(qe[:, 0:wj], ps[:, 0:wj])
                    else:
                        nc.scalar.copy(qe[:, 0:wj], ps[:, 0:wj])
                    nc.sync.dma_start(
                        bass.AP(apad, (128 * t) * RS + j0, [[RS, 128], [1, wj]]),
                        qe[:, 0:wj],
                    )

            # --- stage B: S = QK + bias, exp, scale by 1/rowsum, transposes ---
            for t in range(NT):
                wk_ = 128 * (t + 1)  # causal width
                for hi, bp in enumerate((0, 64)):
                    apad = apads[par * 2 + hi]
                    ppad = ppads[par * 2 + hi]
                    p_sb = sbw.tile([128, 1024], BF16, tag=f"p{hi}", name=f"p{hi}")
                    rs_t = sbw.tile([128, 4], F32, tag=f"rs{hi}", name=f"rs{hi}")
                    ps = pep.tile([128, 1024], F32, tag="mm", name="pss")
                    for (kc0, w) in _chunks(wk_):
                        nc.tensor.matmul(
                            ps[:, kc0:kc0 + w],
                            qt_sb[bp:bp + 64, hp, t * 128:(t + 1) * 128],
                            kt_sb[bp:bp + 64, hp, kc0:kc0 + w],
                            start=True,
                            stop=True,
                        )
                    bia = sbw.tile([128, 1024], BF16, tag="bias")
                    nc.sync.dma_start(
                        bia[:, 0:wk_],
                        bass.AP(
                            apad,
                            (128 * t) * (RS - 1) + 1023,
                            [[RS - 1, 128], [1, wk_]],
                        ),
                    )
                    nc.vector.tensor_add(ps[:, 0:wk_], ps[:, 0:wk_], bia[:, 0:wk_])
                    nc.scalar.activation(
                        p_sb[:, 0:wk_],
                        ps[:, 0:wk_],
                        mybir.ActivationFunctionType.Exp,
                        scale=SCALE,
                        accum_out=rs_t[:, 0:1],
                    )
                    # reciprocal of rowsum -> normalize P in place
                    nc.vector.reciprocal(rs_t[:, 1:2], rs_t[:, 0:1])
                    nc.vector.tensor_scalar_mul(
                        p_sb[:, 0:wk_], p_sb[:, 0:wk_], rs_t[:, 1:2]
                    )
                    # write normalized P rows into ppad (cols 1023..1023+wk_)
                    nc.gpsimd.dma_start(
                        bass.AP(ppad, (128 * t) * RS + 1023, [[RS, 128], [1, wk_]]),
                        p_sb[:, 0:wk_],
                    )
                    # P^T tiles via PE transpose, batched copy into strips
                    trb = trp.tile([128, 8, 128], BF16, tag="tr", name="trb")
                    for u in range(t + 1):
                        nc.tensor.matmul(
                            trb[:, u, :],
                            p_sb[:, u * 128:(u + 1) * 128],
                            id_sb[:],
                            is_transpose=True,
                            start=(u == 0),
                            stop=(u == t),
                        )
                    nc.scalar.copy(
                        pt_s[hi][:, t, 0:t + 1, :], trb[:, 0:t + 1, :]
                    )
            # --- stage B2: PD = diagonally shifted P (read back from ppad) ---
            # separate loop so each read's DMA round trip is hidden behind
            # the other (head, t) iterations of stage B1
            for t in range(NT):
                for hi, bp in enumerate((0, 64)):
                    ppad = ppads[par * 2 + hi]
                    wj = 128 * (t + 1)
                    j0 = L - wj
                    pd_sb = sbw.tile([128, 1024], BF16, tag=f"pd{hi}", name=f"pd{hi}")
                    nc.gpsimd.dma_start(
                        pd_sb[:, 0:wj],
                        bass.AP(
                            ppad, (128 * t) * (RS + 1) + j0, [[RS + 1, 128], [1, wj]]
                        ),
                    )
                    trb2 = trp.tile([128, 8, 128], BF16, tag="tr", name="trb2")
                    for ui in range(t + 1):
                        nc.tensor.matmul(
                            trb2[:, ui, :],
                            pd_sb[:, ui * 128:(ui + 1) * 128],
                            id_sb[:],
                            is_transpose=True,
                            start=(ui == 0),
                            stop=(ui == t),
                        )
                    # tile ui covers j-tile (j0/128 + ui) = 7 - t + ui
                    nc.vector.tensor_copy(
                        pdt_s[hi][:, t, 7 - t:8, :], trb2[:, 0:t + 1, :]
                    )

            # --- stage D: OT = V^T-style PV + E2^T PD accumulation ---
            ot = otp.tile([128, 1024], F32, tag="ot")
            for hi, bp in enumerate((0, 64)):
                # build ordered MM list: (lhsT kind, u, q-range chunks)
                mms = []
                for u in range(8):
                    q0 = 128 * u
                    rngs = [(q0, 512), (512, 1024)] if u < 4 else [(q0, 1024)]
                    for (qa, qb) in rngs:
                        mms.append(("pv", u, qa, qb))
                    jq0 = 128 * (7 - u)
                    rngs = [(jq0, 512), (512, 1024)] if u > 3 else [(jq0, 1024)]
                    for (qa, qb) in rngs:
                        mms.append(("rpe", u, qa, qb))
                # first/last per psum bank (bank0: q<512, bank1: q>=512)
                bank_of = lambda qa: 0 if qa < 512 else 1
                firsts, lasts = {}, {}
                for i, (_, _, qa, _) in enumerate(mms):
                    b_ = bank_of(qa)
                    firsts.setdefault(b_, i)
                    lasts[b_] = i
                for i, (kind, u, qa, qb) in enumerate(mms):
                    b_ = bank_of(qa)
                    if kind == "pv":
                        lhsT = v_sb[:, u, hp * 128 + bp:hp * 128 + bp + 64]
                        # strip tiles (t, u) for t = u..7 cover q in [128u, 1024)
                        rhs = pt_s[hi][:, qa // 128:qb // 128, u, :]
                    else:
                        lhsT = e2_sb[:, u, :]
                        rhs = pdt_s[hi][:, qa // 128:qb // 128, u, :]
                    nc.tensor.matmul(
                        ot[bp:bp + 64, qa:qb],
                        lhsT,
                        rhs,
                        start=(firsts[b_] == i),
                        stop=(lasts[b_] == i),
                        tile_position=(0, bp),
                    )
            nc.vector.tensor_copy(ota_sb[:, hp, :], ot[:])

        attn_ctx.close()

        # ---- output projection ----
        with (
            tc.tile_pool(name="outp_ps", bufs=2, space="PSUM") as ops,
            tc.tile_pool(name="outsb", bufs=2) as osb,
        ):
            ncopy = 0
            for tt in range(8):
                for mc in range(2):
                    ps = ops.tile([128, 512], F32)
                    for hp in range(4):
                        nc.tensor.matmul(
                            ps[:],
                            ota_sb[:, hp, tt * 128:(tt + 1) * 128],
                            wo_sb[:, hp, mc * 512:(mc + 1) * 512],
                            start=(hp == 0),
                            stop=(hp == 3),
                        )
                    ob = osb.tile([128, 512], F32)
                    if ncopy % 2 == 0:
                        nc.vector.tensor_copy(ob[:], ps[:])
                    else:
                        nc.scalar.copy(ob[:], ps[:])
                    ncopy += 1
                    nc.sync.dma_start(
                        outp.ap()[tt * 128:(tt + 1) * 128, mc * 512:(mc + 1) * 512],
                        ob[:],
                    )

    nc.compile()
    return nc


def _get_nc():
    if "nc" not in _CACHE:
        _CACHE["nc"] = _build()
    return _CACHE["nc"]


def kernel(**inputs):
    x = np.asarray(inputs["x"], np.float32)
    Wq = np.asarray(inputs["Wq"], np.float32)
    Wk = np.asarray(inputs["Wk"], np.float32)
    Wv = np.asarray(inputs["Wv"], np.float32)
    Wo = np.asarray(inputs["Wo"], np.float32)
    pek = np.asarray(inputs["pek"], np.float32)
    pev = np.asarray(inputs["pev"], np.float32)

    bf = ml_dtypes.bfloat16
    ek_half = np.ascontiguousarray(pek[1:1 + L].T).astype(bf)   # [64, 1024]
    ekt = np.concatenate([ek_half, ek_half], axis=0)            # [128, 1024]
    e2 = np.ascontiguousarray(pev[1:1 + L]).astype(bf)          # [1024, 64]
    iden = np.eye(128, dtype=np.float32).astype(bf)

    in_maps = []
    for c in range(8):
        b, hg = c >> 1, c & 1
        hs = hg * OD
        in_maps.append(
            dict(
                xt=np.ascontiguousarray(x[b].T).astype(bf),
                wqt=np.ascontiguousarray(Wq[hs:hs + OD].T).astype(bf),
                wkt=np.ascontiguousarray(Wk[hs:hs + OD].T).astype(bf),
                wvt=np.ascontiguousarray(Wv[hs:hs + OD].T).astype(bf),
                wot=np.ascontiguousarray(Wo[:, hs:hs + OD].T).astype(bf),
                ekt=ekt,
                e2=e2,
                iden=iden,
            )
        )

    nc = _get_nc()
    trace = bool(int(os.environ.get("BASS_MHA_TRACE", "0")))
    r = run_bass_kernel_spmd(nc, in_maps, list(range(8)), trace=trace)
    run_info["exec_time_ns"] = r.exec_time_ns
    run_info["profile_json"] = r.profile_json

    out = np.empty((B, L, D), np.float32)
    for b in range(B):
        out[b] = r.results[2 * b]["outp"] + r.results[2 * b + 1]["outp"]
    return out
